# revision 1
# baseline (speedup 1.0000x reference)
"""Context-aware tracker (seq2seq LSTM) Trainium2 kernel.

Model: 50-step encoder LSTM (I=5, H=128) + 12-step autoregressive decoder
LSTM + linear head, B=16384, data-parallel over 8 NeuronCores (2048/core).

Key design points:
  * Layout: hidden dim (128) on SBUF partitions, batch on the free dim.
    Gates are computed as gates.T[512, B] in 4 chunks of 128 rows, each
    chunk one PSUM bank per 512-batch tile:  chunk = Whh_c.T^T @ h  (K=128)
    accumulated with the input part (K=6: 5 input features + ones row that
    carries the bias).
  * The decoder's pred->dec_in feedback is folded into the recurrent
    weights (W_eff = Whh_d + Wih_d[:, :2] @ lin_W), so the linear head is a
    pure output tap off the critical path, and the remaining input part is
    the constant static context (K=4 incl. bias row).
  * Sigmoid over the [i|f|o] chunks is one ACT instruction spanning 3
    contiguous PSUM banks; biases ride in the matmul so no ACT bias needed.
  * i*g runs on GPSIMD to offload the vector engine.
"""

import numpy as np

import concourse.bacc as bacc
import concourse.bass as bass
import concourse.mybir as mybir
import concourse.tile as tile
from concourse.bass_utils import run_bass_kernel_spmd

B, T, I, H, PL = 16384, 50, 5, 128, 12
NCORES = 8
BC = B // NCORES  # 2048 batch per core
NT = 4            # batch tiles per core
TN = BC // NT     # 512 = one PSUM bank of fp32
F32 = mybir.dt.float32
AF = mybir.ActivationFunctionType


def build_nc(n_passes: int = 1) -> bass.Bass:
    # Bacc (not plain Bass): its compile pipeline splits/moves sync waits so
    # every instruction carries at most one (TRN2 constraint walrus enforces).
    nc = bacc.Bacc()
    xt = nc.dram_tensor("xt", [T, 6, BC], F32, kind="ExternalInput")
    w_hh_e = nc.dram_tensor("w_hh_e", [H, 512], F32, kind="ExternalInput")
    w_ih_e = nc.dram_tensor("w_ih_e", [6, 512], F32, kind="ExternalInput")
    w_hh_d = nc.dram_tensor("w_hh_d", [H, 512], F32, kind="ExternalInput")
    w_ih_d0 = nc.dram_tensor("w_ih_d0", [6, 512], F32, kind="ExternalInput")
    w_eff = nc.dram_tensor("w_eff", [H, 512], F32, kind="ExternalInput")
    w_sx = nc.dram_tensor("w_sx", [4, 512], F32, kind="ExternalInput")
    w_lin = nc.dram_tensor("w_lin", [H, 2], F32, kind="ExternalInput")
    out = nc.dram_tensor("out", [PL, 2, BC], F32, kind="ExternalOutput")

    with tile.TileContext(nc) as tc:
        with (
            tc.tile_pool(name="persist", bufs=1) as P1,
            tc.tile_pool(name="state", bufs=2) as PS,
            tc.tile_pool(name="gates", bufs=2) as PG,
            tc.tile_pool(name="psum", bufs=2, space="PSUM") as PP,
        ):
            whh_e = P1.tile([H, 512], F32, tag="whh_e")
            wih_e = P1.tile([6, 512], F32, tag="wih_e")
            whh_d = P1.tile([H, 512], F32, tag="whh_d")
            wih_d0 = P1.tile([6, 512], F32, tag="wih_d0")
            weff = P1.tile([H, 512], F32, tag="weff")
            wsx = P1.tile([4, 512], F32, tag="wsx")
            wlin = P1.tile([H, 2], F32, tag="wlin")
            nc.sync.dma_start(whh_e[:, :], w_hh_e[:, :])
            nc.sync.dma_start(wih_e[:, :], w_ih_e[:, :])
            nc.sync.dma_start(whh_d[:, :], w_hh_d[:, :])
            nc.sync.dma_start(wih_d0[:, :], w_ih_d0[:, :])
            nc.sync.dma_start(weff[:, :], w_eff[:, :])
            nc.sync.dma_start(wsx[:, :], w_sx[:, :])
            nc.sync.dma_start(wlin[:, :], w_lin[:, :])

            stg = [P1.tile([6, BC], F32, tag=f"stg{k}", name=f"stg{k}")
                   for k in range(2)]
            stg49 = P1.tile([6, BC], F32, tag="stg49")
            sxa = P1.tile([4, BC], F32, tag="sxa")
            nc.sync.dma_start(sxa[:, :], xt[T - 1, 2:6, :])

            h_prev = [None] * NT
            c_prev = [None] * NT

            def lstm_step(s, lhsT_h, lhsT_x, xK, rhs_x, dec_t, first):
                # prefetch next encoder x stage
                if s + 1 < T:
                    buf = stg49 if s + 1 == T - 1 else stg[(s + 1) % 2]
                    nc.sync.dma_start(buf[:, :], xt[s + 1, :, :])
                for j in range(NT):
                    sl = bass.ts(j, TN)
                    ps = PP.tile([H, 3, TN], F32, tag="ifo", name="ps")
                    pg = PP.tile([H, TN], F32, tag="g", name="pg")
                    g4 = PG.tile([H, 4, TN], F32, tag=f"G{j}", name="g4")
                    hn = PS.tile([H, TN], F32, tag=f"H{j}", name="hn")
                    cn = PS.tile([H, TN], F32, tag=f"C{j}", name="cn")
                    tc_ = PS.tile([H, TN], F32, tag=f"TC{j}", name="tc_")
                    if not first:
                        hs = h_prev[j][:, :]
                        nc.tensor.matmul(ps[:, 0, :], whh_slc(lhsT_h, 0), hs,
                                         start=True, stop=False)
                        nc.tensor.matmul(ps[:, 1, :], whh_slc(lhsT_h, 1), hs,
                                         start=True, stop=False)
                        nc.tensor.matmul(pg[:, :], whh_slc(lhsT_h, 2), hs,
                                         start=True, stop=False)
                        nc.tensor.matmul(ps[:, 2, :], whh_slc(lhsT_h, 3), hs,
                                         start=True, stop=False)
                    xs = rhs_x[:, sl]
                    nc.tensor.matmul(ps[:, 0, :], lhsT_x[0:xK, 0:128], xs,
                                     start=first, stop=True)
                    nc.tensor.matmul(ps[:, 1, :], lhsT_x[0:xK, 128:256], xs,
                                     start=first, stop=True)
                    nc.tensor.matmul(pg[:, :], lhsT_x[0:xK, 256:384], xs,
                                     start=first, stop=True)
                    nc.tensor.matmul(ps[:, 2, :], lhsT_x[0:xK, 384:512], xs,
                                     start=first, stop=True)
                    # i,f,o in one sigmoid over 3 contiguous banks
                    nc.scalar.activation(g4[:, 0:3, :], ps[:, :, :], AF.Sigmoid)
                    nc.scalar.activation(g4[:, 3, :], pg[:, :], AF.Tanh)
                    if first:
                        nc.gpsimd.tensor_mul(cn[:, :], g4[:, 0, :], g4[:, 3, :])
                    else:
                        # i*g in place (gpsimd), f*c then += i*g on DVE
                        nc.gpsimd.tensor_mul(g4[:, 0, :], g4[:, 0, :],
                                             g4[:, 3, :])
                        nc.vector.tensor_mul(cn[:, :], g4[:, 1, :],
                                             c_prev[j][:, :])
                        nc.vector.tensor_add(cn[:, :], cn[:, :], g4[:, 0, :])
                    nc.scalar.activation(tc_[:, :], cn[:, :], AF.Tanh)
                    nc.vector.tensor_mul(hn[:, :], g4[:, 2, :], tc_[:, :])
                    if dec_t is not None:
                        pp = PP.tile([H, TN], F32, tag="g", name="pp")
                        nc.tensor.matmul(pp[0:2, :], wlin[:, :], hn[:, :],
                                         start=True, stop=True)
                        pb = PS.tile([2, TN], F32, tag=f"PB{j}", name="pb")
                        nc.vector.tensor_copy(pb[:, :], pp[0:2, :])
                        nc.sync.dma_start(out[dec_t, :, sl], pb[:, :])
                    h_prev[j], c_prev[j] = hn, cn

            def whh_slc(w, c):
                return w[:, bass.ts(c, 128)]

            for _ in range(n_passes):
                nc.sync.dma_start(stg[0][:, :], xt[0, :, :])
                # ---- encoder ----
                for t in range(T):
                    rx = stg49 if t == T - 1 else stg[t % 2]
                    lstm_step(t, whh_e, wih_e, 6, rx, None, t == 0)
                # ---- decoder step 0 (dec_in = x[:, -1, :]) ----
                lstm_step(T, whh_d, wih_d0, 6, stg49, 0, False)
                # ---- decoder steps 1..11 (feedback folded into weff) ----
                for t in range(1, PL):
                    lstm_step(T + t, weff, wsx, 4, sxa, t, False)

    nc.finalize()
    return nc


def prep_inputs(inputs):
    x = np.asarray(inputs["x"], np.float32)
    enc_Wih = np.asarray(inputs["enc_Wih"], np.float32)
    enc_Whh = np.asarray(inputs["enc_Whh"], np.float32)
    enc_bih = np.asarray(inputs["enc_bih"], np.float32)
    enc_bhh = np.asarray(inputs["enc_bhh"], np.float32)
    dec_Wih = np.asarray(inputs["dec_Wih"], np.float32)
    dec_Whh = np.asarray(inputs["dec_Whh"], np.float32)
    dec_bih = np.asarray(inputs["dec_bih"], np.float32)
    dec_bhh = np.asarray(inputs["dec_bhh"], np.float32)
    lin_W = np.asarray(inputs["lin_W"], np.float32)
    lin_b = np.asarray(inputs["lin_b"], np.float32)

    bias_e = enc_bih + enc_bhh
    w_ih_e = np.ascontiguousarray(
        np.concatenate([enc_Wih.T, bias_e[None, :]], axis=0))
    bias_d0 = dec_bih + dec_bhh
    w_ih_d0 = np.ascontiguousarray(
        np.concatenate([dec_Wih.T, bias_d0[None, :]], axis=0))
    w_eff = np.ascontiguousarray((dec_Whh + dec_Wih[:, 0:2] @ lin_W).T)
    bias_d = bias_d0 + dec_Wih[:, 0:2] @ lin_b
    w_sx = np.ascontiguousarray(
        np.concatenate([dec_Wih[:, 2:5].T, bias_d[None, :]], axis=0))
    shared = {
        "w_hh_e": np.ascontiguousarray(enc_Whh.T),
        "w_ih_e": w_ih_e,
        "w_hh_d": np.ascontiguousarray(dec_Whh.T),
        "w_ih_d0": w_ih_d0,
        "w_eff": w_eff,
        "w_sx": w_sx,
        "w_lin": np.ascontiguousarray(lin_W.T),
    }
    in_maps = []
    for c in range(NCORES):
        xc = x[c * BC : (c + 1) * BC]          # [BC, T, I]
        xt = np.empty((T, 6, BC), np.float32)
        xt[:, 0:I, :] = xc.transpose(1, 2, 0)
        xt[:, I, :] = 1.0
        in_maps.append({"xt": xt, **shared})
    return in_maps, lin_b


def assemble(outs, lin_b):
    preds = np.empty((B, PL, 2), np.float32)
    for c in range(NCORES):
        o = np.asarray(outs[c])  # [PL, 2, BC]
        preds[c * BC : (c + 1) * BC] = o.transpose(2, 0, 1)
    preds += lin_b.reshape(1, 1, 2)
    return preds


def kernel(**inputs) -> np.ndarray:
    in_maps, lin_b = prep_inputs(inputs)
    nc = build_nc()
    res = run_bass_kernel_spmd(nc, in_maps, core_ids=list(range(NCORES)))
    return assemble([r["out"] for r in res.results], lin_b)



# revision 11
# speedup vs baseline: 80.4846x; 80.4846x over previous
"""Context-aware tracker (seq2seq LSTM) Trainium2 kernel.

Model: 50-step encoder LSTM (I=5, H=128) + 12-step autoregressive decoder
LSTM + linear head, B=16384, data-parallel over 8 NeuronCores (2048/core).

Key design points:
  * Layout: hidden dim (128) on SBUF partitions, batch on the free dim.
    Gates are computed as gates.T[512, B] in 4 chunks of 128 rows, each
    chunk one PSUM bank per 512-batch tile:  chunk = Whh_c.T^T @ h  (K=128)
    accumulated with the input part (K=6: 5 input features + ones row that
    carries the bias).
  * bf16 weights/activations (PSUM accumulation stays fp32): matmul runs at
    1 cycle/row (vs 4 for fp32) and DVE elementwise gets 2x packed mode.
    The cell state c stays fp32 to bound accumulation error over 62 steps.
  * The decoder's pred->dec_in feedback is folded into the recurrent
    weights (W_eff = Whh_d + Wih_d[:, :2] @ lin_W), so the linear head is a
    pure output tap off the critical path, and the remaining input part is
    the constant static context (K=4 incl. bias row).
  * Sigmoid over the [i|f|o] chunks is one ACT instruction spanning 3
    contiguous PSUM banks; biases ride in the matmul so no ACT bias needed.
  * i*g runs on GPSIMD to offload the vector engine.
"""

import numpy as np
import ml_dtypes

import concourse.bacc as bacc
import concourse.bass as bass
import concourse.mybir as mybir
import concourse.tile as tile
from concourse.bass_utils import run_bass_kernel_spmd

B, T, I, H, PL = 16384, 50, 5, 128, 12
NCORES = 8
BC = B // NCORES  # 2048 batch per core
NT = 4            # batch tiles per core
TN = BC // NT     # 512 = one PSUM bank of fp32
F32 = mybir.dt.float32
BF16 = mybir.dt.bfloat16
AF = mybir.ActivationFunctionType
BF = ml_dtypes.bfloat16


def build_nc(n_passes: int = 1) -> bass.Bass:
    # Bacc (not plain Bass): its compile pipeline splits/moves sync waits so
    # every instruction carries at most one (TRN2 constraint walrus enforces).
    nc = bacc.Bacc()
    xt = nc.dram_tensor("xt", [T, 6, BC], BF16, kind="ExternalInput")
    w_hh_e = nc.dram_tensor("w_hh_e", [H, 512], BF16, kind="ExternalInput")
    w_ih_e = nc.dram_tensor("w_ih_e", [6, 512], BF16, kind="ExternalInput")
    w_hh_d = nc.dram_tensor("w_hh_d", [H, 512], BF16, kind="ExternalInput")
    w_ih_d0 = nc.dram_tensor("w_ih_d0", [6, 512], BF16, kind="ExternalInput")
    w_eff = nc.dram_tensor("w_eff", [H, 512], BF16, kind="ExternalInput")
    w_sx = nc.dram_tensor("w_sx", [4, 512], BF16, kind="ExternalInput")
    w_lin = nc.dram_tensor("w_lin", [H, 2], BF16, kind="ExternalInput")
    out = nc.dram_tensor("out", [PL, 2, BC], F32, kind="ExternalOutput")

    with tile.TileContext(nc) as tc:
        with (
            tc.tile_pool(name="persist", bufs=1) as P1,
            tc.tile_pool(name="state", bufs=2) as PS,
            tc.tile_pool(name="gates", bufs=2) as PG,
            tc.tile_pool(name="psum", bufs=2, space="PSUM") as PP,
        ):
            whh_e = P1.tile([H, 512], BF16, tag="whh_e")
            wih_e = P1.tile([6, 512], BF16, tag="wih_e")
            whh_d = P1.tile([H, 512], BF16, tag="whh_d")
            wih_d0 = P1.tile([6, 512], BF16, tag="wih_d0")
            weff = P1.tile([H, 512], BF16, tag="weff")
            wsx = P1.tile([4, 512], BF16, tag="wsx")
            wlin = P1.tile([H, 2], BF16, tag="wlin")
            nc.sync.dma_start(whh_e[:, :], w_hh_e[:, :])
            nc.sync.dma_start(wih_e[:, :], w_ih_e[:, :])
            nc.sync.dma_start(whh_d[:, :], w_hh_d[:, :])
            nc.sync.dma_start(wih_d0[:, :], w_ih_d0[:, :])
            nc.sync.dma_start(weff[:, :], w_eff[:, :])
            nc.sync.dma_start(wsx[:, :], w_sx[:, :])
            nc.sync.dma_start(wlin[:, :], w_lin[:, :])

            stg = [P1.tile([6, BC], BF16, tag=f"stg{k}", name=f"stg{k}")
                   for k in range(2)]
            stg49 = P1.tile([6, BC], BF16, tag="stg49")
            sxa = P1.tile([4, BC], BF16, tag="sxa")
            nc.sync.dma_start(sxa[:, :], xt[T - 1, 2:6, :])

            h_prev = [None] * NT
            c_prev = [None] * NT
            # Deferred "tail" stages (tanh(c), h = o*tanh(c), decoder tap),
            # emitted one batch-tile later so ACT's strict FIFO never stalls
            # on the DVE cell chain: while tile j's cell state settles, ACT
            # is busy with tile j+1's sigmoid/tanh.
            pending = []

            def emit_tail():
                if pending:
                    pending.pop(0)()

            def lstm_step(s, lhsT_h, lhsT_x, xK, rhs_x, dec_t, first):
                # prefetch next encoder x stage
                if s + 1 < T:
                    buf = stg49 if s + 1 == T - 1 else stg[(s + 1) % 2]
                    nc.sync.dma_start(buf[:, :], xt[s + 1, :, :])
                for j in range(NT):
                    sl = bass.ts(j, TN)
                    ps = PP.tile([H, 3, TN], F32, tag="ifo", name="ps")
                    pg = PP.tile([H, TN], F32, tag="g", name="pg")
                    g4 = PG.tile([H, 4, TN], BF16, tag=f"G{j}", name="g4")
                    hn = PS.tile([H, TN], BF16, tag=f"H{j}", name="hn")
                    cn = PS.tile([H, TN], F32, tag=f"C{j}", name="cn")
                    tc_ = PS.tile([H, TN], BF16, tag=f"TC{j}", name="tc_")
                    if not first:
                        hs = h_prev[j][:, :]
                        nc.tensor.matmul(ps[:, 0, :], whh_slc(lhsT_h, 0), hs,
                                         start=True, stop=False)
                        nc.tensor.matmul(ps[:, 1, :], whh_slc(lhsT_h, 1), hs,
                                         start=True, stop=False)
                        nc.tensor.matmul(pg[:, :], whh_slc(lhsT_h, 2), hs,
                                         start=True, stop=False)
                        nc.tensor.matmul(ps[:, 2, :], whh_slc(lhsT_h, 3), hs,
                                         start=True, stop=False)
                    xs = rhs_x[:, sl]
                    nc.tensor.matmul(ps[:, 0, :], lhsT_x[0:xK, 0:128], xs,
                                     start=first, stop=True)
                    nc.tensor.matmul(ps[:, 1, :], lhsT_x[0:xK, 128:256], xs,
                                     start=first, stop=True)
                    nc.tensor.matmul(pg[:, :], lhsT_x[0:xK, 256:384], xs,
                                     start=first, stop=True)
                    nc.tensor.matmul(ps[:, 2, :], lhsT_x[0:xK, 384:512], xs,
                                     start=first, stop=True)
                    # i,f,o in one sigmoid over 3 contiguous banks
                    nc.scalar.activation(g4[:, 0:3, :], ps[:, :, :], AF.Sigmoid)
                    nc.scalar.activation(g4[:, 3, :], pg[:, :], AF.Tanh)
                    if first:
                        nc.vector.tensor_mul(cn[:, :], g4[:, 0, :], g4[:, 3, :])
                    else:
                        # all cell math on DVE: with bf16 the packed 2x mode
                        # makes i*g cheap, and keeping it off GPSIMD removes
                        # the slowest engine from the recurrent chain
                        nc.vector.tensor_mul(g4[:, 0, :], g4[:, 0, :],
                                             g4[:, 3, :])
                        nc.vector.tensor_mul(cn[:, :], g4[:, 1, :],
                                             c_prev[j][:, :])
                        nc.vector.tensor_add(cn[:, :], cn[:, :], g4[:, 0, :])

                    def tail(g4=g4, hn=hn, cn=cn, tc_=tc_, sl=sl, j=j,
                             dec_t=dec_t):
                        nc.scalar.activation(tc_[:, :], cn[:, :], AF.Tanh)
                        nc.vector.tensor_mul(hn[:, :], g4[:, 2, :], tc_[:, :])
                        if dec_t is not None:
                            pp = PP.tile([H, TN], F32, tag="g", name="pp")
                            nc.tensor.matmul(pp[0:2, :], wlin[:, :], hn[:, :],
                                             start=True, stop=True)
                            pb = PS.tile([2, TN], F32, tag=f"PB{j}",
                                         name="pb")
                            nc.vector.tensor_copy(pb[:, :], pp[0:2, :])
                            nc.sync.dma_start(out[dec_t, :, sl], pb[:, :])

                    # pop the PREVIOUS tile's tail first (skew = 1 tile),
                    # then enqueue this tile's tail.
                    emit_tail()
                    pending.append(tail)
                    h_prev[j], c_prev[j] = hn, cn

            def whh_slc(w, c):
                return w[:, bass.ts(c, 128)]

            for _ in range(n_passes):
                nc.sync.dma_start(stg[0][:, :], xt[0, :, :])
                # ---- encoder ----
                for t in range(T):
                    rx = stg49 if t == T - 1 else stg[t % 2]
                    lstm_step(t, whh_e, wih_e, 6, rx, None, t == 0)
                # ---- decoder step 0 (dec_in = x[:, -1, :]) ----
                lstm_step(T, whh_d, wih_d0, 6, stg49, 0, False)
                # ---- decoder steps 1..11 (feedback folded into weff) ----
                for t in range(1, PL):
                    lstm_step(T + t, weff, wsx, 4, sxa, t, False)
                # flush the last deferred tail of this pass
                emit_tail()

    nc.finalize()
    return nc


def prep_inputs(inputs):
    x = np.asarray(inputs["x"], np.float32)
    enc_Wih = np.asarray(inputs["enc_Wih"], np.float32)
    enc_Whh = np.asarray(inputs["enc_Whh"], np.float32)
    enc_bih = np.asarray(inputs["enc_bih"], np.float32)
    enc_bhh = np.asarray(inputs["enc_bhh"], np.float32)
    dec_Wih = np.asarray(inputs["dec_Wih"], np.float32)
    dec_Whh = np.asarray(inputs["dec_Whh"], np.float32)
    dec_bih = np.asarray(inputs["dec_bih"], np.float32)
    dec_bhh = np.asarray(inputs["dec_bhh"], np.float32)
    lin_W = np.asarray(inputs["lin_W"], np.float32)
    lin_b = np.asarray(inputs["lin_b"], np.float32)

    bias_e = enc_bih + enc_bhh
    w_ih_e = np.ascontiguousarray(
        np.concatenate([enc_Wih.T, bias_e[None, :]], axis=0))
    bias_d0 = dec_bih + dec_bhh
    w_ih_d0 = np.ascontiguousarray(
        np.concatenate([dec_Wih.T, bias_d0[None, :]], axis=0))
    w_eff = np.ascontiguousarray((dec_Whh + dec_Wih[:, 0:2] @ lin_W).T)
    bias_d = bias_d0 + dec_Wih[:, 0:2] @ lin_b
    w_sx = np.ascontiguousarray(
        np.concatenate([dec_Wih[:, 2:5].T, bias_d[None, :]], axis=0))
    shared = {
        "w_hh_e": np.ascontiguousarray(enc_Whh.T).astype(BF),
        "w_ih_e": w_ih_e.astype(BF),
        "w_hh_d": np.ascontiguousarray(dec_Whh.T).astype(BF),
        "w_ih_d0": w_ih_d0.astype(BF),
        "w_eff": w_eff.astype(BF),
        "w_sx": w_sx.astype(BF),
        "w_lin": np.ascontiguousarray(lin_W.T).astype(BF),
    }
    in_maps = []
    for c in range(NCORES):
        xc = x[c * BC : (c + 1) * BC]          # [BC, T, I]
        xt = np.empty((T, 6, BC), BF)
        xt[:, 0:I, :] = xc.transpose(1, 2, 0).astype(BF)
        xt[:, I, :] = 1.0
        in_maps.append({"xt": xt, **shared})
    return in_maps, lin_b


def assemble(outs, lin_b):
    preds = np.empty((B, PL, 2), np.float32)
    for c in range(NCORES):
        o = np.asarray(outs[c])  # [PL, 2, BC]
        preds[c * BC : (c + 1) * BC] = o.transpose(2, 0, 1)
    preds += lin_b.reshape(1, 1, 2)
    return preds


def kernel(**inputs) -> np.ndarray:
    in_maps, lin_b = prep_inputs(inputs)
    nc = build_nc()
    res = run_bass_kernel_spmd(nc, in_maps, core_ids=list(range(NCORES)))
    return assemble([r["out"] for r in res.results], lin_b)
